# revision 26
# baseline (speedup 1.0000x reference)
"""Trainium2 Bass kernel for BondingGraphGNN (gnn_message_passing), v3.

Model (see reference):
  h = relu(x @ W_emb)
  4x: m = h @ W_msg[i]; agg = scatter_add(m[src] -> dst); h = GRU(agg, h)
  h = relu(h); pooled = segment_mean(h, batch); out = softplus(relu(pooled@W1+b1)@W2+b2)

v3 design:
  Nodes are graph-aligned-sharded across the 8 cores; edges live with their
  DST core. Per step each core computes its local message shard (bf16,
  node-major) and an AllGather with Shared output assembles the full table
  in shared HBM (cheap on-chip: each core writes 1.7MB + barrier). Each core
  then gathers rows for its incoming edges with dma_gather spread over 4
  SWDGE queues (the gather is descriptor-rate-bound; queue parallelism is
  worth ~6x), using window-packed tiles: per (dst-512-chunk, lo/hi half of
  the int16 index space) a compile-time window sequence shared by all cores
  packs edges into ~dense 128-edge tiles. Aggregation is
  matmul(lhsT=E_tile, rhs=S_onehot) accumulated at window offsets into a
  [128, 512] PSUM chunk, which is already feature-major: it feeds the GRU
  directly with no transpose, no staging, and no reduce collective.
"""

import os
import numpy as np
import ml_dtypes

import sys as _sys, types as _types
try:
    import antenv.axon_hooks  # noqa: F401
except Exception:
    _m = _types.ModuleType("antenv.axon_hooks")
    _m.get_axon_ntff_profile_hook = lambda: None
    _sys.modules["antenv.axon_hooks"] = _m

import concourse.bacc as bacc
import concourse.bass as bass
import concourse.mybir as mybir
import concourse.tile as tile
from concourse.bass_utils import run_bass_kernel_spmd

F32 = mybir.dt.float32
BF16 = mybir.dt.bfloat16
I16 = mybir.dt.int16
AF = mybir.ActivationFunctionType

N_NODES = 50000
N_EDGES = 800000
FEAT = 90
H = 128
STEPS = 4
N_GRAPHS = 100
N_CORES = 8
G_PAD = 16
CHUNK = 512
PAD_SLOT = 255.0

LAST_RESULTS = {}


# ----------------------------------------------------------------------------
# host-side layout
# ----------------------------------------------------------------------------

def _preprocess(x, edge_index, batch):
    batch = np.asarray(batch, np.int64)
    src = np.asarray(edge_index[0], np.int64)
    dst = np.asarray(edge_index[1], np.int64)

    counts = np.bincount(batch, minlength=N_GRAPHS)
    cum = np.concatenate([[0], np.cumsum(counts)])
    targets = [round(N_NODES * c / N_CORES) for c in range(N_CORES + 1)]
    gsplit = [0]
    for c in range(1, N_CORES):
        g = int(np.argmin(np.abs(cum - targets[c])))
        g = max(g, gsplit[-1])
        gsplit.append(g)
    gsplit.append(N_GRAPHS)
    bounds = np.array([cum[g] for g in gsplit], np.int64)
    n_c = np.diff(bounds)
    assert (n_c > 0).all()
    n_graphs_c = np.diff(np.array(gsplit))
    assert n_graphs_c.max() <= G_PAD

    n_pad = int(np.ceil(n_c.max() / CHUNK) * CHUNK)
    CH = n_pad // CHUNK
    table_rows = N_CORES * n_pad
    table_split = table_rows // 2
    assert table_split <= 32767 and table_rows - table_split <= 32767

    core_of = np.searchsorted(bounds, np.arange(N_NODES), side="right") - 1

    c_e = core_of[dst]                     # edge -> owning core (by dst)
    d_loc = dst - bounds[c_e]
    j_e = d_loc // CHUNK
    rel_e = d_loc % CHUNK
    tpos = core_of[src] * n_pad + (src - bounds[core_of[src]])
    half_e = (tpos >= table_split).astype(np.int64)
    iv_e = np.where(half_e == 0, tpos, tpos - table_split)

    # ---- window-packed tiles per (j, half): shared compile-time window
    # sequence; each core fills tiles lockstep with <=128 edges whose rel
    # falls in [W_k, W_k+128).
    order0 = np.lexsort((rel_e, half_e, j_e, c_e))
    ec, ej, eh = c_e[order0], j_e[order0], half_e[order0]
    erel = rel_e[order0]
    eidx = iv_e[order0]
    grp_key = (ec * CH + ej) * 2 + eh
    grp_cnt = np.bincount(grp_key, minlength=N_CORES * CH * 2)
    grp_off = np.concatenate([[0], np.cumsum(grp_cnt)])
    place_tile = np.empty(len(order0), np.int64)
    place_row = np.empty(len(order0), np.int64)
    place_slot = np.empty(len(order0), np.int64)

    windows = [[None, None] for _ in range(CH)]
    B = np.zeros((CH, 2), np.int64)
    tbh = np.zeros((CH, 2), np.int64)      # first tile of (j, half)
    t = 0
    for j in range(CH):
        for hh in range(2):
            tbh[j, hh] = t
            segs = []
            for c in range(N_CORES):
                gk = (c * CH + j) * 2 + hh
                segs.append((grp_off[gk], grp_off[gk + 1]))
            ptr = [s for s, _ in segs]
            W = []
            while True:
                live = [c for c in range(N_CORES) if ptr[c] < segs[c][1]]
                if not live:
                    break
                w = min(int(erel[ptr[c]]) for c in live)
                w = min(w, CHUNK - 128)
                k = len(W)
                W.append(w)
                for c in live:
                    s0, s1 = ptr[c], segs[c][1]
                    hi = np.searchsorted(erel[s0:s1], w + 128, side="left")
                    take = min(int(hi), 128)
                    if take:
                        sel = slice(s0, s0 + take)
                        place_tile[sel] = t + k
                        place_row[sel] = np.arange(take)
                        place_slot[sel] = erel[sel] - w
                        ptr[c] = s0 + take
            if not W:
                W = [0]
            windows[j][hh] = W
            B[j, hh] = len(W)
            t += len(W)
    t_tot = t

    idx_arr = np.zeros((N_CORES, 16, t_tot * 8), np.int16)
    slot_arr = np.full((N_CORES, 128, t_tot), PAD_SLOT, np.float32)
    gpos = (place_tile - tbh[ej, eh]) * 128 + place_row
    idx_arr[ec, gpos % 16, tbh[ej, eh] * 8 + gpos // 16] = eidx.astype(np.int16)
    slot_arr[ec, place_row, place_tile] = place_slot

    per_core = []
    nt = n_pad // 128
    for c in range(N_CORES):
        nc_nodes = int(n_c[c])
        xT = np.zeros((FEAT, n_pad), ml_dtypes.bfloat16)
        xT[:, :nc_nodes] = np.asarray(x[bounds[c]:bounds[c + 1]], np.float32).T
        gloc = (batch[bounds[c]:bounds[c + 1]] - gsplit[c]).astype(np.int64)
        gmat = np.zeros((128, nt * G_PAD), np.float32)
        node_ids = np.arange(nc_nodes)
        gmat[node_ids % 128, (node_ids // 128) * G_PAD + gloc] = 1.0
        cc = counts[gsplit[c]:gsplit[c + 1]].astype(np.float32)
        invc = np.zeros((G_PAD, 1), np.float32)
        invc[:len(cc), 0] = 1.0 / np.maximum(cc, 1.0)
        per_core.append(dict(
            xT=xT,
            idx=np.tile(idx_arr[c], (8, 1)),
            slot=slot_arr[c],
            gmat=gmat,
            invc=invc,
        ))

    meta = dict(n_pad=n_pad, CH=CH, t_tot=t_tot, B=B, windows=windows,
                tbh=tbh, table_rows=table_rows, table_split=table_split,
                bounds=bounds, gsplit=gsplit, n_graphs_c=n_graphs_c)
    return per_core, meta


# ----------------------------------------------------------------------------
# device program
# ----------------------------------------------------------------------------

def _build(meta):
    DBG_STEPS = int(os.environ.get("K_STEPS", STEPS))
    NO_AG = bool(int(os.environ.get("K_NO_AG", "0")))
    NO_GATHER = bool(int(os.environ.get("K_NO_GATHER", "0")))
    GSP = int(os.environ.get("K_GSP", "4"))
    NQ = int(os.environ.get("K_NQ", "4"))
    n_pad = meta["n_pad"]
    CH = meta["CH"]
    t_tot = meta["t_tot"]
    B = meta["B"]
    windows = meta["windows"]
    tbh = meta["tbh"]
    table_rows = meta["table_rows"]
    table_split = meta["table_split"]
    nt = n_pad // 128
    NG = CHUNK // 128

    nc = bacc.Bacc("TRN2", target_bir_lowering=False, debug=False,
                   num_devices=N_CORES, num_swdge_queues=NQ)

    d_xT = nc.dram_tensor("xT", [FEAT, n_pad], BF16, kind="ExternalInput")
    d_idx = nc.dram_tensor("idx", [128, t_tot * 8], I16, kind="ExternalInput")
    d_slot = nc.dram_tensor("slot", [128, t_tot], F32, kind="ExternalInput")
    d_gmat = nc.dram_tensor("gmat", [128, nt * G_PAD], F32, kind="ExternalInput")
    d_invc = nc.dram_tensor("invc", [G_PAD, 1], F32, kind="ExternalInput")
    d_iota = nc.dram_tensor("iota", [1, 128], F32, kind="ExternalInput")
    d_ident = nc.dram_tensor("ident", [128, 128], F32, kind="ExternalInput")
    d_wemb = nc.dram_tensor("wemb", [FEAT, H], F32, kind="ExternalInput")
    d_wmsg = nc.dram_tensor("wmsg", [STEPS, H, H], F32, kind="ExternalInput")
    d_wih = nc.dram_tensor("wih", [H, 3 * H], F32, kind="ExternalInput")
    d_whh = nc.dram_tensor("whh", [H, 3 * H], F32, kind="ExternalInput")
    d_bihT = nc.dram_tensor("bihT", [H, 3], F32, kind="ExternalInput")
    d_bhhT = nc.dram_tensor("bhhT", [H, 3], F32, kind="ExternalInput")
    d_w1 = nc.dram_tensor("w1", [H, H], F32, kind="ExternalInput")
    d_b1 = nc.dram_tensor("b1", [H, 1], F32, kind="ExternalInput")
    d_w2 = nc.dram_tensor("w2", [H, 1], F32, kind="ExternalInput")
    d_b2 = nc.dram_tensor("b2", [1, 1], F32, kind="ExternalInput")
    d_out = nc.dram_tensor("out", [1, G_PAD], F32, kind="ExternalOutput")

    with tile.TileContext(nc) as tc:
        with (
            tc.tile_pool(name="persist", bufs=1) as P,
            tc.tile_pool(name="dram", bufs=1, space="DRAM") as DR,
            tc.tile_pool(name="stgw", bufs=2) as STW,
            tc.tile_pool(name="xpool", bufs=2) as XP,
            tc.tile_pool(name="epool", bufs=4) as EP,
            tc.tile_pool(name="spool", bufs=6) as SP,
            tc.tile_pool(name="atp", bufs=3) as ATP,
            tc.tile_pool(name="gpool", bufs=2) as GP,
            tc.tile_pool(name="mpool", bufs=4) as MP,
            tc.tile_pool(name="ps_agg", bufs=2, space="PSUM") as PS_AGG,
            tc.tile_pool(name="ps_gru", bufs=4, space="PSUM") as PS_GRU,
            tc.tile_pool(name="ps_m", bufs=2, space="PSUM") as PS_M,
        ):
            PS_TR = PS_M
            shards = [DR.tile([n_pad, H], BF16, name=f"shard{s}")
                      for s in range(STEPS)]
            tables = [DR.tile([table_rows, H], BF16, addr_space="Shared",
                              name=f"table{s}") for s in range(STEPS)]

            # ---------------- constants / weights ----------------
            slot_all = P.tile([128, t_tot], F32, name="slot_all")
            nc.sync.dma_start(out=slot_all[:], in_=d_slot[:, :])
            idx_all = P.tile([128, t_tot * 8], I16, name="idx_all")
            nc.sync.dma_start(out=idx_all[:], in_=d_idx[:, :])

            iota_f = STW.tile([128, 128], F32, name="iota_f", tag="stgf")
            nc.sync.dma_start(out=iota_f[:], in_=d_iota.ap().to_broadcast([128, 128]))
            iota_b = P.tile([128, 128], BF16, name="iota_b")
            nc.vector.tensor_copy(iota_b[:], iota_f[:])

            ident_f = STW.tile([128, 128], F32, name="ident_f", tag="stgf")
            nc.sync.dma_start(out=ident_f[:], in_=d_ident[:, :])
            ident_b = P.tile([128, 128], BF16, name="ident_b")
            nc.vector.tensor_copy(ident_b[:], ident_f[:])

            def load_bf(dram_ap, shape, name):
                tf = STW.tile(shape, F32, name=name + "_f", tag="stgf")
                nc.sync.dma_start(out=tf[:], in_=dram_ap)
                tb_ = P.tile(shape, BF16, name=name + "_b")
                nc.scalar.activation(tb_[:], tf[:], AF.Copy)
                return tb_

            wemb_b = load_bf(d_wemb[:, :], [FEAT, H], "wemb")
            wmsg_b = [load_bf(d_wmsg[s, :, :], [H, H], f"wmsg{s}")
                      for s in range(STEPS)]
            wih_b = load_bf(d_wih[:, :], [H, 3 * H], "wih")
            whh_b = load_bf(d_whh[:, :], [H, 3 * H], "whh")
            w1_b = load_bf(d_w1[:, :], [H, H], "w1")
            w2_b = load_bf(d_w2[:, :], [H, 1], "w2")

            bih = P.tile([H, 3], F32, name="bih")
            nc.sync.dma_start(out=bih[:], in_=d_bihT[:, :])
            bhh = P.tile([H, 3], F32, name="bhh")
            nc.sync.dma_start(out=bhh[:], in_=d_bhhT[:, :])
            bsum = P.tile([H, 3], F32, name="bsum")
            nc.vector.tensor_add(bsum[:], bih[:], bhh[:])
            b1t = P.tile([H, 1], F32, name="b1t")
            nc.sync.dma_start(out=b1t[:], in_=d_b1[:, :])
            b2t = P.tile([1, 1], F32, name="b2t")
            nc.sync.dma_start(out=b2t[:], in_=d_b2[:, :])
            invc_t = P.tile([G_PAD, 1], F32, name="invc_t")
            nc.sync.dma_start(out=invc_t[:], in_=d_invc[:, :])
            gmat_f = STW.tile([128, nt * G_PAD], F32, name="gmat_f", tag="stgf")
            nc.sync.dma_start(out=gmat_f[:], in_=d_gmat[:, :])
            gmat_b = P.tile([128, nt * G_PAD], BF16, name="gmat_b")
            nc.scalar.activation(gmat_b[:], gmat_f[:], AF.Copy)
            zeros_c = P.tile([128, CHUNK], BF16, name="zeros_c")
            nc.vector.memset(zeros_c[:], 0.0)

            # state
            hA = P.tile([128, n_pad], BF16, name="hA")
            hB = P.tile([128, n_pad], BF16, name="hB")
            m_all = P.tile([128, n_pad], BF16, name="m_all")

            def emit_m(step, h_tile, j):
                """messages for chunk j -> m_all -> shard rows."""
                sl = slice(j * CHUNK, (j + 1) * CHUNK)
                for k in range(NG):
                    c0 = j * CHUNK + k * 128
                    pm = PS_M.tile([128, 128], F32, name="pm", tag="pmisc")
                    nc.tensor.matmul(pm[:], lhsT=h_tile[:, c0:c0 + 128],
                                     rhs=wmsg_b[step][:, :], start=True, stop=True)
                    nc.scalar.activation(m_all[:, c0:c0 + 128], pm[:], AF.Copy)
                nc.sync.dma_start(
                    out=shards[step][j * CHUNK:(j + 1) * CHUNK, :]
                        .rearrange("(a p) b -> p a b", p=128),
                    in_=m_all[:, sl].rearrange("p (a b) -> p a b", b=128))

            def all_gather(step):
                if NO_AG:
                    return
                nc.gpsimd.collective_compute(
                    "AllGather", mybir.AluOpType.bypass,
                    ins=[shards[step].opt()], outs=[tables[step].opt()],
                    replica_groups=[list(range(N_CORES))],
                )

            # ---------------- embedding + m0 ----------------
            for j in range(CH):
                sl = slice(j * CHUNK, (j + 1) * CHUNK)
                xT_b = XP.tile([FEAT, CHUNK], BF16, name="xT_b", tag="stgx")
                nc.sync.dma_start(out=xT_b[:], in_=d_xT[:, sl])
                pe = PS_GRU.tile([128, CHUNK], F32, name="pe_emb", tag="pgru")
                nc.tensor.matmul(pe[:], lhsT=wemb_b[:, :], rhs=xT_b[:, :],
                                 start=True, stop=True)
                nc.scalar.activation(hA[:, sl], pe[:], AF.Relu)
                emit_m(0, hA, j)
            all_gather(0)

            # ---------------- message-passing steps ----------------
            for step in range(DBG_STEPS):
                h_cur = hA if step % 2 == 0 else hB
                h_nxt = hB if step % 2 == 0 else hA
                table = tables[step]
                tab_ap = [table[0:table_split, :], table[table_split:table_rows, :]]

                for j in range(CH):
                    sl = slice(j * CHUNK, (j + 1) * CHUNK)
                    # gather both halves for this chunk
                    E_h = []
                    for hh in range(2):
                        tj = int(B[j, hh])
                        t0 = int(tbh[j, hh])
                        E_t = EP.tile([128, tj, 128], BF16, name="E", tag="E")
                        E_h.append(E_t)
                        if NO_GATHER:
                            continue
                        splits = np.linspace(0, tj, GSP + 1).astype(int)
                        for si in range(GSP):
                            a, b_ = int(splits[si]), int(splits[si + 1])
                            if a == b_:
                                continue
                            nc.gpsimd.dma_gather(
                                E_t[:, a:b_, :], tab_ap[hh],
                                idx_all[:, (t0 + a) * 8:(t0 + b_) * 8],
                                (b_ - a) * 128, (b_ - a) * 128, H,
                                single_packet=False,
                                queue_num=((j * 2 + hh) * GSP + si) % NQ)
                    # aggregate into [128, CHUNK] psum (feature-major)
                    pa = PS_AGG.tile([128, CHUNK], F32, name="pa", tag="pa")
                    nc.tensor.matmul(pa[:], lhsT=ident_b[:], rhs=zeros_c[:],
                                     start=True, stop=False)
                    n_all = int(B[j, 0] + B[j, 1])
                    ki = 0
                    for hh in range(2):
                        t0 = int(tbh[j, hh])
                        for k in range(int(B[j, hh])):
                            tg = t0 + k
                            w = int(windows[j][hh][k])
                            St = SP.tile([128, 128], BF16, name="St", tag="St")
                            nc.vector.tensor_scalar(
                                St[:], iota_b[:], slot_all[:, tg:tg + 1], None,
                                mybir.AluOpType.is_equal)
                            ki += 1
                            if NO_GATHER:
                                continue
                            nc.tensor.matmul(pa[:, w:w + 128],
                                             lhsT=E_h[hh][:, k, :], rhs=St[:],
                                             start=False, stop=(ki == n_all),
                                             skip_group_check=True)
                    aT = ATP.tile([128, CHUNK], BF16, name="aT", tag="aT")
                    nc.scalar.activation(aT[:], pa[:], AF.Copy)

                    # GRU for chunk j
                    hb_j = h_cur[:, sl]
                    p_r = PS_GRU.tile([128, CHUNK], F32, name="p_r", tag="pgru")
                    nc.tensor.matmul(p_r[:], lhsT=wih_b[:, 0:128], rhs=aT[:],
                                     start=True, stop=False)
                    nc.tensor.matmul(p_r[:], lhsT=whh_b[:, 0:128], rhs=hb_j,
                                     start=False, stop=True)
                    p_z = PS_GRU.tile([128, CHUNK], F32, name="p_z", tag="pgru")
                    nc.tensor.matmul(p_z[:], lhsT=wih_b[:, 128:256], rhs=aT[:],
                                     start=True, stop=False)
                    nc.tensor.matmul(p_z[:], lhsT=whh_b[:, 128:256], rhs=hb_j,
                                     start=False, stop=True)
                    p_xn = PS_GRU.tile([128, CHUNK], F32, name="p_xn", tag="pgru")
                    nc.tensor.matmul(p_xn[:], lhsT=wih_b[:, 256:384], rhs=aT[:],
                                     start=True, stop=True)
                    p_hn = PS_GRU.tile([128, CHUNK], F32, name="p_hn", tag="pgru")
                    nc.tensor.matmul(p_hn[:], lhsT=whh_b[:, 256:384], rhs=hb_j,
                                     start=True, stop=True)
                    r_t = GP.tile([128, CHUNK], BF16, name="r_t")
                    nc.scalar.activation(r_t[:], p_r[:], AF.Sigmoid, bias=bsum[:, 0:1])
                    z_t = GP.tile([128, CHUNK], BF16, name="z_t")
                    nc.scalar.activation(z_t[:], p_z[:], AF.Sigmoid, bias=bsum[:, 1:2])
                    hn_t = GP.tile([128, CHUNK], BF16, name="hn_t")
                    nc.scalar.activation(hn_t[:], p_hn[:], AF.Identity,
                                         bias=bhh[:, 2:3])
                    t1 = GP.tile([128, CHUNK], BF16, name="t1")
                    nc.vector.tensor_mul(t1[:], r_t[:], hn_t[:])
                    u_t = GP.tile([128, CHUNK], F32, name="u_t")
                    nc.vector.tensor_add(u_t[:], t1[:], p_xn[:])
                    n_t = GP.tile([128, CHUNK], F32, name="n_t")
                    nc.scalar.activation(n_t[:], u_t[:], AF.Tanh, bias=bih[:, 2:3])
                    d_t = GP.tile([128, CHUNK], F32, name="d_t")
                    nc.vector.tensor_sub(d_t[:], h_cur[:, sl], n_t[:])
                    e_t = GP.tile([128, CHUNK], F32, name="e_t")
                    nc.vector.tensor_mul(e_t[:], z_t[:], d_t[:])
                    nc.vector.tensor_add(h_nxt[:, sl], n_t[:], e_t[:])
                    if step < DBG_STEPS - 1:
                        emit_m(step + 1, h_nxt, j)
                if step < DBG_STEPS - 1:
                    all_gather(step + 1)

            # ---------------- readout ----------------
            h_fin = hA if DBG_STEPS % 2 == 0 else hB
            h_rel = hB if DBG_STEPS % 2 == 0 else hA
            for j in range(CH):
                sl = slice(j * CHUNK, (j + 1) * CHUNK)
                nc.scalar.activation(h_rel[:, sl], h_fin[:, sl], AF.Relu)
            pp = PS_GRU.tile([G_PAD, 128], F32, name="pp", tag="pgru")
            for t in range(nt):
                hb = h_rel[:, t * 128:(t + 1) * 128]
                ptr2 = PS_TR.tile([128, 128], BF16, name="ptr2", tag="pmisc")
                nc.tensor.transpose(ptr2[:], hb, ident_b[:])
                hnm = MP.tile([128, 128], BF16, name="hnm", tag="mp")
                nc.scalar.activation(hnm[:], ptr2[:], AF.Copy)
                nc.tensor.matmul(pp[:], lhsT=gmat_b[:, t * G_PAD:(t + 1) * G_PAD],
                                 rhs=hnm[:], start=(t == 0), stop=(t == nt - 1))
            pooled = P.tile([G_PAD, 128], BF16, name="pooled")
            nc.vector.tensor_scalar(pooled[:], pp[:], invc_t[:], None,
                                    mybir.AluOpType.mult)
            ppt = PS_TR.tile([128, G_PAD], BF16, name="ppt", tag="pmisc")
            nc.tensor.transpose(ppt[:], pooled[:], ident_b[0:G_PAD, 0:G_PAD])
            pooledT = P.tile([128, G_PAD], BF16, name="pooledT")
            nc.scalar.activation(pooledT[:], ppt[:], AF.Copy)
            pz1 = PS_M.tile([128, G_PAD], F32, name="pz1", tag="pmisc")
            nc.tensor.matmul(pz1[:], lhsT=w1_b[:, :], rhs=pooledT[:],
                             start=True, stop=True)
            z1 = P.tile([128, G_PAD], BF16, name="z1")
            nc.scalar.activation(z1[:], pz1[:], AF.Relu, bias=b1t[:, 0:1])
            po = PS_M.tile([1, G_PAD], F32, name="po", tag="pmisc")
            nc.tensor.matmul(po[:], lhsT=w2_b[:, :], rhs=z1[:],
                             start=True, stop=True)
            esb = P.tile([1, G_PAD], F32, name="esb")
            nc.scalar.activation(esb[:], po[:], AF.Exp, bias=b2t[:, 0:1])
            osb = P.tile([1, G_PAD], F32, name="osb")
            nc.scalar.activation(osb[:], esb[:], AF.Ln, bias=1.0)
            nc.sync.dma_start(out=d_out[:, :], in_=osb[:])

    nc.compile()
    return nc


# ----------------------------------------------------------------------------
# entry point
# ----------------------------------------------------------------------------

def kernel(x, edge_index, batch, W_emb, W_msg, W_ih, W_hh, b_ih, b_hh,
           W1, b1, W2, b2):
    x = np.asarray(x, np.float32)
    per_core, meta = _preprocess(x, edge_index, batch)
    nc = _build(meta)

    shared = dict(
        iota=np.arange(128, dtype=np.float32).reshape(1, 128),
        ident=np.eye(128, dtype=np.float32),
        wemb=np.asarray(W_emb, np.float32),
        wmsg=np.asarray(W_msg, np.float32),
        wih=np.asarray(W_ih, np.float32),
        whh=np.asarray(W_hh, np.float32),
        bihT=np.ascontiguousarray(np.asarray(b_ih, np.float32).reshape(3, H).T),
        bhhT=np.ascontiguousarray(np.asarray(b_hh, np.float32).reshape(3, H).T),
        w1=np.asarray(W1, np.float32),
        b1=np.asarray(b1, np.float32).reshape(H, 1),
        w2=np.asarray(W2, np.float32),
        b2=np.asarray(b2, np.float32).reshape(1, 1),
    )
    in_maps = []
    for c in range(N_CORES):
        m = dict(shared)
        for k in ("xT", "idx", "slot", "gmat", "invc"):
            m[k] = per_core[c][k]
        in_maps.append(m)

    trace = bool(int(os.environ.get("KERNEL_TRACE", "0")))
    res = run_bass_kernel_spmd(nc, in_maps, list(range(N_CORES)), trace=trace)
    LAST_RESULTS["exec_time_ns"] = res.exec_time_ns
    LAST_RESULTS["profile_json"] = res.profile_json
    LAST_RESULTS["nc"] = nc
    LAST_RESULTS["in_maps"] = in_maps

    out = np.zeros((N_GRAPHS,), np.float32)
    gsplit = meta["gsplit"]
    for c in range(N_CORES):
        ng = gsplit[c + 1] - gsplit[c]
        out[gsplit[c]:gsplit[c + 1]] = res.results[c]["out"][0, :ng]
    return out
